# revision 29
# baseline (speedup 1.0000x reference)
"""AttentionBlock (GroupNorm + single-head spatial attention + residual) on 8
Trainium2 NeuronCores.

Sharding: pure data-parallel over batch — B=32 images, 4 per core, no
collectives. Full C=512 weights are replicated to every core.

Per-core kernel (per batch item), all layouts chosen so NO on-device
transposes are ever needed:
  x   [C=512 part(4x128), N=1024 free]  (f32)
  GroupNorm stats: per-channel bn_stats, then 16-channel group reduction /
    broadcast via tiny indicator matmuls (contraction along partitions).
  h = x*A + B (one ACT op per c-tile, per-partition scale/bias) -> f32r
  q,k  [C part, N free] = WqT.T @ h   (PE, f32r) + bias (ACT copy)
  vT   [N part, C free] = h.T @ WvT   (PE, f32r; h used as stationary)
  scoresT[j,i] = k.T q                (PE, accumulate over 4 c-tiles)
  expT = exp(scale*scoresT)           (ACT, psum->sbuf f32r; no max-sub:
                                       scores are ~N(0,1), |s|<10)
  U[c,i] = vT.T @ expT, sumexp[i] = ones.T @ expT  (PE, accumulated over j,
                                       ct-outer so PSUM->SBUF casts overlap)
  F = WpT.T @ U                       (PE)
  out = x + F * (1/sumexp)_bcast + pb_eff   (DVE)
where pb_eff = proj_b + proj_w @ bv folds the v-bias through attention
(rows of softmax sum to 1) and the projection.

float32r = fp32 with mantissa RNE-rounded to 11 bits (verified bit-exact vs
host rounding): matmul runs at 1 cycle/row (4x faster than fp32) with
~1.2e-4 element rounding as the only error source.

The group-norm stats for batch b+1 are emitted in the middle of batch b's
last score loop so the bn_stats land on the otherwise-idle Vector engine
and the tiny stats matmuls never stall the PE.
"""

import numpy as np

import concourse.bacc as bacc
import concourse.bass as bass
import concourse.tile as tile
from concourse.tile import add_dep_helper
from concourse import mybir
from concourse.bass_utils import run_bass_kernel_spmd

F32 = mybir.dt.float32
F32R = mybir.dt.float32r

B_TOTAL = 32
N_CORES = 8
B_CORE = B_TOTAL // N_CORES  # 4
C = 512
N = 1024  # H*W
G = 32  # groups
GS = C // G  # 16 channels per group
CT = C // 128  # 4 c-tiles
JT = N // 128  # 8 j-tiles
IH = N // 512  # 2 i-halves
EPS = 1e-5
SCALE = float(C) ** -0.5


def round_fp32r(x: np.ndarray) -> np.ndarray:
    """fp32 -> e8m11 (RNE on bit 12); matches device rounding bit-exactly."""
    u = x.view(np.uint32).astype(np.uint64)
    u = (u + 0x7FF + ((u >> 12) & 1)) & ~np.uint64(0xFFF)
    return u.astype(np.uint32).view(np.float32)


def build_program() -> bass.Bass:
    nc = bacc.Bacc("TRN2", target_bir_lowering=False)

    x_d = nc.declare_dram_parameter("x", [B_CORE, 128, CT, N], F32, isOutput=False)
    wq_d = nc.declare_dram_parameter("wq", [128, CT, C], F32R, isOutput=False)
    wk_d = nc.declare_dram_parameter("wk", [128, CT, C], F32R, isOutput=False)
    wv_d = nc.declare_dram_parameter("wv", [128, CT, C], F32R, isOutput=False)
    wp_d = nc.declare_dram_parameter("wp", [128, CT, C], F32R, isOutput=False)
    qb_d = nc.declare_dram_parameter("qb_t", [128, 2 * CT], F32, isOutput=False)
    nw_d = nc.declare_dram_parameter("nw_t", [128, CT], F32, isOutput=False)
    nb_d = nc.declare_dram_parameter("nb_t", [128, CT], F32, isOutput=False)
    pbe_d = nc.declare_dram_parameter("pbe_t", [128, CT], F32, isOutput=False)
    e_d = nc.declare_dram_parameter("E", [128, 8], F32, isOutput=False)
    et_d = nc.declare_dram_parameter("ET", [8, 128], F32, isOutput=False)
    out_d = nc.declare_dram_parameter("out", [B_CORE, 128, CT, N], F32, isOutput=True)

    with tile.TileContext(nc) as tc:
        with (
            tc.tile_pool(name="weights", bufs=1) as wpool,
            tc.tile_pool(name="xin", bufs=2) as xpool,
            tc.tile_pool(name="hbuf", bufs=1) as hpool,
            tc.tile_pool(name="qkv", bufs=1) as qkvpool,
            tc.tile_pool(name="expb", bufs=12) as epool,
            tc.tile_pool(name="usb", bufs=1) as upool,
            tc.tile_pool(name="outb", bufs=1) as opool,
            tc.tile_pool(name="stat", bufs=2) as spool,
            tc.tile_pool(name="mm512", bufs=4, space="PSUM") as mmps,
            tc.tile_pool(name="ups", bufs=2, space="PSUM") as ups,
            tc.tile_pool(name="sums", bufs=1, space="PSUM") as sums,
            tc.tile_pool(name="statps", bufs=1, space="PSUM") as statps,
        ):
            # ---- one-time loads (tiny params first; big weights are
            # streamed in usage order AFTER x so the stats/h chain and the
            # first q matmuls are not stuck behind 4MB of weight DMA) ----
            qb_t = wpool.tile([128, 2 * CT], F32, tag="qb")
            nc.sync.dma_start(out=qb_t, in_=qb_d[:, :])
            nw_t = wpool.tile([128, CT], F32, tag="nw")
            nc.sync.dma_start(out=nw_t, in_=nw_d[:, :])
            nb_t = wpool.tile([128, CT], F32, tag="nb")
            nc.sync.dma_start(out=nb_t, in_=nb_d[:, :])
            pbe_t = wpool.tile([128, CT], F32, tag="pbe")
            nc.sync.dma_start(out=pbe_t, in_=pbe_d[:, :])
            e_t = wpool.tile([128, 8], F32, tag="E")
            nc.sync.dma_start(out=e_t, in_=e_d[:, :])
            et_t = wpool.tile([8, 128], F32, tag="ET")
            nc.sync.dma_start(out=et_t, in_=et_d[:, :])
            ones_f = wpool.tile([128, 1], F32, tag="onesf")
            nc.vector.memset(ones_f, 1.0)
            ones_t = wpool.tile([128, 1], F32R, tag="ones")
            nc.vector.tensor_copy(out=ones_t, in_=ones_f)
            eps_t = wpool.tile([8, 1], F32, tag="eps")
            nc.vector.memset(eps_t, EPS)
            # pre-load ACT function tables so the first real Sqrt/Exp/
            # Identity doesn't eat a 1.3us ACT_TABLE_LOAD on the critical path
            warm_t = wpool.tile([1, 4], F32, tag="warm")
            nc.vector.memset(warm_t, 1.0)
            for wf in (
                mybir.ActivationFunctionType.Sqrt,
                mybir.ActivationFunctionType.Exp,
                mybir.ActivationFunctionType.Identity,
            ):
                nc.scalar.activation(
                    out=warm_t[:, 1:2], in_=warm_t[:, 0:1], func=wf,
                    bias=0.0, scale=1.0,
                )
            wq_t = wpool.tile([128, CT, C], F32R, tag="wq")
            wk_t = wpool.tile([128, CT, C], F32R, tag="wk")
            wv_t = wpool.tile([128, CT, C], F32R, tag="wv")
            wp_t = wpool.tile([128, CT, C], F32R, tag="wp")

            x_dma_insts: list = []

            def load_weight(dst, dram):
                w_inst = nc.sync.dma_start(out=dst, in_=dram[:, :, :])
                # keep batch-0 x DMAs ahead of weight bytes in the queues
                for xi in x_dma_insts:
                    add_dep_helper(w_inst.ins, xi.ins, sync=True)

            def stats_a(b, fast=False):
                """x load + per-channel stats + group reduce + finalize.

                fast=True (first batch): per-channel sums via ACT accum and
                sum-of-squares via DVE reduce, running the two engines in
                parallel instead of the serial bn_stats chain; scratch
                outputs land in the not-yet-used h/q buffer slots.
                """
                x_t = xpool.tile([128, CT, N], F32, tag="x")
                x_dma_insts.clear()
                for cth in range(2):
                    cs = slice(cth * 2, cth * 2 + 2)
                    x_dma_insts.append(
                        nc.sync.dma_start(out=x_t[:, cs, :], in_=x_d[b][:, cs, :])
                    )
                mvall = spool.tile([128, CT, 2], F32, tag="mvall")
                if fast:
                    scr_a = hpool.tile([128, CT, N], F32, tag="h", name="scr_a")
                    scr_v = qkvpool.tile([128, CT, N], F32, tag="q", name="scr_v")
                    for ct in range(CT):
                        nc.scalar.activation(
                            out=scr_a[:, ct, :],
                            in_=x_t[:, ct, :],
                            func=mybir.ActivationFunctionType.Square,
                            accum_out=mvall[:, ct, 1:2],
                        )
                        nc.vector.tensor_scalar(
                            out=scr_v[:, ct, :],
                            in0=x_t[:, ct, :],
                            scalar1=0.0,
                            scalar2=0.0,
                            op0=mybir.AluOpType.add,
                            op1=mybir.AluOpType.add,
                            accum_out=mvall[:, ct, 0:1],
                        )
                else:
                    mvsq = spool.tile([128, CT], F32, tag="mvsq")
                    for ct in range(CT):
                        st = spool.tile([128, 2, 6], F32, tag="bnstats")
                        for h2 in range(2):
                            nc.vector.bn_stats(
                                out=st[:, h2, :],
                                in_=x_t[:, ct, h2 * 512 : (h2 + 1) * 512],
                            )
                        nc.vector.bn_aggr(out=mvall[:, ct, :], in_=st)
                    # mvall[:, :, 1] = var + mean^2 = E[x^2]
                    nc.vector.tensor_mul(mvsq, mvall[:, :, 0], mvall[:, :, 0])
                    nc.vector.tensor_add(mvall[:, :, 1], mvall[:, :, 1], mvsq)
                # group sums over 16-partition groups
                gsum_ps = statps.tile([8, CT, 2], F32, tag="statps")
                for ct in range(CT):
                    nc.tensor.matmul(
                        out=gsum_ps[:, ct, :],
                        lhsT=e_t,
                        rhs=mvall[:, ct, :],
                        start=True,
                        stop=True,
                    )
                s_t = spool.tile([8, CT, 2], F32, tag="sstat")
                nc.scalar.mul(
                    out=s_t, in_=gsum_ps, mul=(1.0 / (GS * N)) if fast else 1.0 / GS
                )
                ssq = spool.tile([8, CT], F32, tag="ssq")
                nc.vector.tensor_mul(ssq, s_t[:, :, 0], s_t[:, :, 0])
                nc.vector.tensor_sub(s_t[:, :, 1], s_t[:, :, 1], ssq)
                nc.scalar.activation(
                    out=s_t[:, :, 1],
                    in_=s_t[:, :, 1],
                    func=mybir.ActivationFunctionType.Sqrt,
                    bias=eps_t,
                    scale=1.0,
                )
                nc.vector.reciprocal(out=s_t[:, :, 1], in_=s_t[:, :, 1])
                return x_t, s_t

            def stats_b(x_t, s_t):
                """broadcast (mean, rstd) to channels; h = x*A + B (f32r)."""
                bc_ps = statps.tile([128, CT, 2], F32, tag="statps")
                for ct in range(CT):
                    nc.tensor.matmul(
                        out=bc_ps[:, ct, :],
                        lhsT=et_t,
                        rhs=s_t[:, ct, :],
                        start=True,
                        stop=True,
                    )
                ab_t = spool.tile([128, 2, CT], F32, tag="ab")
                tmp_t = spool.tile([128, CT], F32, tag="abtmp")
                for ct in range(CT):
                    nc.vector.tensor_mul(
                        ab_t[:, 0, ct : ct + 1],
                        bc_ps[:, ct, 1:2],
                        nw_t[:, ct : ct + 1],
                    )
                    nc.vector.tensor_scalar_mul(
                        tmp_t[:, ct : ct + 1],
                        bc_ps[:, ct, 0:1],
                        ab_t[:, 0, ct : ct + 1],
                    )
                    nc.vector.tensor_sub(
                        ab_t[:, 1, ct : ct + 1],
                        nb_t[:, ct : ct + 1],
                        tmp_t[:, ct : ct + 1],
                    )
                h_t = hpool.tile([128, CT, N], F32R, tag="h")
                for ct in range(CT):
                    if ct < 2:
                        nc.scalar.activation(
                            out=h_t[:, ct, :],
                            in_=x_t[:, ct, :],
                            func=mybir.ActivationFunctionType.Identity,
                            bias=ab_t[:, 1, ct : ct + 1],
                            scale=ab_t[:, 0, ct : ct + 1],
                        )
                    else:
                        nc.vector.tensor_scalar(
                            out=h_t[:, ct, :],
                            in0=x_t[:, ct, :],
                            scalar1=ab_t[:, 0, ct : ct + 1],
                            scalar2=ab_t[:, 1, ct : ct + 1],
                            op0=mybir.AluOpType.mult,
                            op1=mybir.AluOpType.add,
                        )
                return h_t

            st = stats_a(0, fast=True)
            # weights stream behind x, in first-use order
            load_weight(wk_t, wk_d)
            load_weight(wq_t, wq_d)
            load_weight(wv_t, wv_d)
            load_weight(wp_t, wp_d)
            cur_x, cur_h = st[0], stats_b(*st)
            next_ctx: list = [None]

            for b in range(B_CORE):
                x_t, h_t = cur_x, cur_h

                # ---- q, k projections: [c part, i free] ----
                q_t = qkvpool.tile([128, CT, N], F32R, tag="q")
                k_t = qkvpool.tile([128, CT, N], F32R, tag="k")

                def qkv_chunk(dst, w_t, ct, ih, bias_col, on_act):
                    isl = slice(ih * 512, (ih + 1) * 512)
                    pp = mmps.tile([128, 512], F32, tag="mm", name="pp")
                    for kt in range(CT):
                        nc.tensor.matmul(
                            out=pp,
                            lhsT=w_t[:, kt, ct * 128 : (ct + 1) * 128],
                            rhs=h_t[:, kt, isl],
                            start=(kt == 0),
                            stop=(kt == CT - 1),
                        )
                    if on_act and ct < 2:
                        nc.scalar.activation(
                            out=dst[:, ct, isl],
                            in_=pp,
                            func=mybir.ActivationFunctionType.Identity,
                            bias=qb_t[:, bias_col : bias_col + 1],
                            scale=1.0,
                        )
                    else:
                        # bias-add + f32r round on DVE, keeping ACT free
                        nc.vector.tensor_scalar_add(
                            dst[:, ct, isl], pp, qb_t[:, bias_col : bias_col + 1]
                        )

                # k fully (scores need all j columns); q first half only —
                # the second half is produced during the ih0 score loop
                for ct in range(CT):
                    for ih in range(IH):
                        qkv_chunk(k_t, wk_t, ct, ih, CT + ct, on_act=False)
                for ct in range(CT):
                    qkv_chunk(q_t, wq_t, ct, 0, ct, on_act=True)
                # ---- v in transposed layout: vT [j part, c free] ----
                vt_t = qkvpool.tile([128, JT, C], F32R, tag="vt")
                for jt in range(JT):
                    vp = mmps.tile([128, 512], F32, tag="mm")
                    for kt in range(CT):
                        nc.tensor.matmul(
                            out=vp,
                            lhsT=h_t[:, kt, jt * 128 : (jt + 1) * 128],
                            rhs=wv_t[:, kt, :],
                            start=(kt == 0),
                            stop=(kt == CT - 1),
                        )
                    nc.vector.tensor_copy(out=vt_t[:, jt, :], in_=vp)

                # ---- attention ----
                out_t = opool.tile([128, CT, N], F32, tag="out")
                for ih in range(IH):
                    last_stage = ih == IH - 1
                    isl = slice(ih * 512, (ih + 1) * 512)
                    se_ps = sums.tile([1, 512], F32, tag="se")
                    e_list = []
                    for jt in range(JT):
                        sc_ps = mmps.tile([128, 512], F32, tag="mm")
                        for kt in range(CT):
                            nc.tensor.matmul(
                                out=sc_ps,
                                lhsT=k_t[:, kt, jt * 128 : (jt + 1) * 128],
                                rhs=q_t[:, kt, isl],
                                start=(kt == 0),
                                stop=(kt == CT - 1),
                            )
                        e_sb = epool.tile([128, 512], F32R, tag="exp")
                        nc.scalar.activation(
                            out=e_sb,
                            in_=sc_ps,
                            func=mybir.ActivationFunctionType.Exp,
                            bias=0.0,
                            scale=SCALE,
                        )
                        e_list.append(e_sb)
                        if jt >= 2:
                            nc.tensor.matmul(
                                out=se_ps,
                                lhsT=ones_t,
                                rhs=e_list[jt - 2],
                                start=(jt == 2),
                                stop=False,
                            )
                        # hoist deferred work into this PE-dense stretch:
                        # ih0: produce q's second half; ih1: next batch stats
                        if ih == 0 and 1 <= jt <= CT:
                            qkv_chunk(q_t, wq_t, jt - 1, 1, jt - 1, on_act=True)
                        if last_stage and b + 1 < B_CORE:
                            if jt == 3:
                                next_ctx[0] = stats_a(b + 1)
                            elif jt == 6:
                                nx, ns = next_ctx[0]
                                next_ctx[0] = (nx, stats_b(nx, ns))
                    for jtl in (JT - 2, JT - 1):
                        nc.tensor.matmul(
                            out=se_ps,
                            lhsT=ones_t,
                            rhs=e_list[jtl],
                            start=False,
                            stop=(jtl == JT - 1),
                        )
                    # ct-outer AV accumulation; casts overlap next ct's MMs.
                    # The [1,512] reciprocal (3.3us on one partition) is
                    # split into [1,128] chunks interleaved between the
                    # casts so it never head-of-line-blocks the DVE FIFO.
                    rec_sb = upool.tile([1, 512], F32, tag="rec")
                    rb_t = upool.tile([128, 512], F32, tag="rb")
                    u_sb = upool.tile([128, CT, 512], F32R, tag="usb")
                    for ct in range(CT):
                        u_ps = ups.tile([128, 512], F32, tag="u")
                        for jt in range(JT):
                            nc.tensor.matmul(
                                out=u_ps,
                                lhsT=vt_t[:, jt, ct * 128 : (ct + 1) * 128],
                                rhs=e_list[jt],
                                start=(jt == 0),
                                stop=(jt == JT - 1),
                            )
                        nc.vector.tensor_copy(out=u_sb[:, ct, :], in_=u_ps)
                        csl = slice(ct * 128, (ct + 1) * 128)
                        nc.vector.reciprocal(
                            out=rec_sb[:, csl], in_=se_ps[:, csl]
                        )
                        nc.gpsimd.partition_broadcast(
                            rb_t[:, csl], rec_sb[:, csl]
                        )
                    # ---- proj + epilogue ----
                    for ct in range(CT):
                        f_ps = mmps.tile([128, 512], F32, tag="mm")
                        for kt in range(CT):
                            nc.tensor.matmul(
                                out=f_ps,
                                lhsT=wp_t[:, kt, ct * 128 : (ct + 1) * 128],
                                rhs=u_sb[:, kt, :],
                                start=(kt == 0),
                                stop=(kt == CT - 1),
                            )
                        m_sb = epool.tile([128, 512], F32, tag="exp")
                        nc.vector.tensor_mul(m_sb, f_ps, rb_t)
                        nc.vector.scalar_tensor_tensor(
                            out=out_t[:, ct, isl],
                            in0=m_sb,
                            scalar=pbe_t[:, ct : ct + 1],
                            in1=x_t[:, ct, isl],
                            op0=mybir.AluOpType.add,
                            op1=mybir.AluOpType.add,
                        )
                    if b == B_CORE - 1:
                        # last batch: fire each half as soon as it's done
                        nc.sync.dma_start(
                            out=out_d[b][:, :, isl], in_=out_t[:, :, isl]
                        )
                if b < B_CORE - 1:
                    nc.sync.dma_start(out=out_d[b][:, :, :], in_=out_t)
                if b + 1 < B_CORE:
                    cur_x, cur_h = next_ctx[0]
                    next_ctx[0] = None
    nc.compile()
    return nc


_NC_CACHE: list[bass.Bass | None] = [None]


def kernel(
    x: np.ndarray,
    norm_w: np.ndarray,
    norm_b: np.ndarray,
    qkv_w: np.ndarray,
    qkv_b: np.ndarray,
    proj_w: np.ndarray,
    proj_b: np.ndarray,
) -> np.ndarray:
    in_maps = prepare_in_maps(x, norm_w, norm_b, qkv_w, qkv_b, proj_w, proj_b)
    if _NC_CACHE[0] is None:
        _NC_CACHE[0] = build_program()
    nc = _NC_CACHE[0]
    res = run_bass_kernel_spmd(nc, in_maps, list(range(N_CORES)))
    return assemble_output(res.results, x.shape)


def prepare_in_maps(x, norm_w, norm_b, qkv_w, qkv_b, proj_w, proj_b):
    x = np.ascontiguousarray(np.asarray(x, dtype=np.float32))
    norm_w = np.asarray(norm_w, dtype=np.float32)
    norm_b = np.asarray(norm_b, dtype=np.float32)
    qkv_w = np.asarray(qkv_w, dtype=np.float32)
    qkv_b = np.asarray(qkv_b, dtype=np.float32)
    proj_w = np.asarray(proj_w, dtype=np.float32)
    proj_b = np.asarray(proj_b, dtype=np.float32)

    Bf, Cf, H, W = x.shape
    assert (Bf, Cf, H * W) == (B_TOTAL, C, N)

    # weights pre-transposed + packed (128, CT, 512) so each SBUF partition
    # loads one contiguous 8KB run, and pre-rounded to fp32r
    wT = round_fp32r(np.ascontiguousarray(qkv_w.T))  # (C_in, 3C)
    wT = wT.reshape(CT, 128, 3 * C).transpose(1, 0, 2)  # (128, CT, 3C)
    wq = np.ascontiguousarray(wT[:, :, :C])
    wk = np.ascontiguousarray(wT[:, :, C : 2 * C])
    wv = np.ascontiguousarray(wT[:, :, 2 * C :])
    wpT = round_fp32r(np.ascontiguousarray(proj_w.T))
    wp = np.ascontiguousarray(wpT.reshape(CT, 128, C).transpose(1, 0, 2))

    qb_t = np.ascontiguousarray(qkv_b[: 2 * C].reshape(2 * CT, 128).T)
    nw_t = np.ascontiguousarray(norm_w.reshape(CT, 128).T)
    nb_t = np.ascontiguousarray(norm_b.reshape(CT, 128).T)
    pb_eff = (
        proj_b.astype(np.float64) + proj_w.astype(np.float64) @ qkv_b[2 * C :]
    ).astype(np.float32)
    pbe_t = np.ascontiguousarray(pb_eff.reshape(CT, 128).T)
    pidx = np.arange(128)
    E = (pidx[:, None] // GS == np.arange(8)[None, :]).astype(np.float32)
    ET = np.ascontiguousarray(E.T)

    # x packed (B, 128, CT, N): 16KB contiguous per partition per batch
    xp = np.ascontiguousarray(
        x.reshape(B_TOTAL, CT, 128, N).transpose(0, 2, 1, 3)
    )
    shared = {
        "wq": wq,
        "wk": wk,
        "wv": wv,
        "wp": wp,
        "qb_t": qb_t,
        "nw_t": nw_t,
        "nb_t": nb_t,
        "pbe_t": pbe_t,
        "E": E,
        "ET": ET,
    }
    return [
        {"x": np.ascontiguousarray(xp[c * B_CORE : (c + 1) * B_CORE]), **shared}
        for c in range(N_CORES)
    ]


def assemble_output(results, x_shape):
    Bf, Cf, H, W = x_shape
    out = np.concatenate([results[c]["out"] for c in range(N_CORES)], axis=0)
    # (B, 128, CT, N) -> (B, C, H, W)
    out = out.transpose(0, 2, 1, 3).reshape(B_TOTAL, C, H, W)
    return np.ascontiguousarray(out)


# revision 30
# speedup vs baseline: 1.2316x; 1.2316x over previous
"""AttentionBlock (GroupNorm + single-head spatial attention + residual) on 8
Trainium2 NeuronCores.

Sharding: pure data-parallel over batch — B=32 images, 4 per core, no
collectives. Full C=512 weights are replicated to every core.

Per-core kernel (per batch item), all layouts chosen so NO on-device
transposes are ever needed:
  x   [C=512 part(4x128), N=1024 free]  (f32)
  GroupNorm stats: per-channel bn_stats, then 16-channel group reduction /
    broadcast via tiny indicator matmuls (contraction along partitions).
  h = x*A + B (one ACT op per c-tile, per-partition scale/bias) -> f32r
  q,k  [C part, N free] = WqT.T @ h   (PE, f32r) + bias (ACT copy)
  vT   [N part, C free] = h.T @ WvT   (PE, f32r; h used as stationary)
  scoresT[j,i] = k.T q                (PE, accumulate over 4 c-tiles)
  expT = exp(scale*scoresT)           (ACT, psum->sbuf f32r; no max-sub:
                                       scores are ~N(0,1), |s|<10)
  U[c,i] = vT.T @ expT, sumexp[i] = ones.T @ expT  (PE, accumulated over j,
                                       ct-outer so PSUM->SBUF casts overlap)
  F = WpT.T @ U                       (PE)
  out = x + F * (1/sumexp)_bcast + pb_eff   (DVE)
where pb_eff = proj_b + proj_w @ bv folds the v-bias through attention
(rows of softmax sum to 1) and the projection.

float32r = fp32 with mantissa RNE-rounded to 11 bits (verified bit-exact vs
host rounding): matmul runs at 1 cycle/row (4x faster than fp32) with
~1.2e-4 element rounding as the only error source.

The group-norm stats for batch b+1 are emitted in the middle of batch b's
last score loop so the bn_stats land on the otherwise-idle Vector engine
and the tiny stats matmuls never stall the PE.
"""

import numpy as np

import concourse.bacc as bacc
import concourse.bass as bass
import concourse.tile as tile
from concourse.tile import add_dep_helper
from concourse import mybir
from concourse.bass_utils import run_bass_kernel_spmd

F32 = mybir.dt.float32
F32R = mybir.dt.float32r

B_TOTAL = 32
N_CORES = 8
B_CORE = B_TOTAL // N_CORES  # 4
C = 512
N = 1024  # H*W
G = 32  # groups
GS = C // G  # 16 channels per group
CT = C // 128  # 4 c-tiles
JT = N // 128  # 8 j-tiles
IH = N // 512  # 2 i-halves
EPS = 1e-5
SCALE = float(C) ** -0.5


def round_fp32r(x: np.ndarray) -> np.ndarray:
    """fp32 -> e8m11 (RNE on bit 12); matches device rounding bit-exactly."""
    u = x.view(np.uint32).astype(np.uint64)
    u = (u + 0x7FF + ((u >> 12) & 1)) & ~np.uint64(0xFFF)
    return u.astype(np.uint32).view(np.float32)


def build_program() -> bass.Bass:
    nc = bacc.Bacc("TRN2", target_bir_lowering=False)

    x_d = nc.declare_dram_parameter("x", [B_CORE, 128, CT, N], F32, isOutput=False)
    wq_d = nc.declare_dram_parameter("wq", [128, CT, C], F32R, isOutput=False)
    wk_d = nc.declare_dram_parameter("wk", [128, CT, C], F32R, isOutput=False)
    wv_d = nc.declare_dram_parameter("wv", [128, CT, C], F32R, isOutput=False)
    wp_d = nc.declare_dram_parameter("wp", [128, CT, C], F32R, isOutput=False)
    qb_d = nc.declare_dram_parameter("qb_t", [128, 2 * CT], F32, isOutput=False)
    nw_d = nc.declare_dram_parameter("nw_t", [128, CT], F32, isOutput=False)
    nb_d = nc.declare_dram_parameter("nb_t", [128, CT], F32, isOutput=False)
    pbe_d = nc.declare_dram_parameter("pbe_t", [128, CT], F32, isOutput=False)
    e_d = nc.declare_dram_parameter("E", [128, 8], F32, isOutput=False)
    et_d = nc.declare_dram_parameter("ET", [8, 128], F32, isOutput=False)
    out_d = nc.declare_dram_parameter("out", [B_CORE, 128, CT, N], F32, isOutput=True)

    with tile.TileContext(nc) as tc:
        with (
            tc.tile_pool(name="weights", bufs=1) as wpool,
            tc.tile_pool(name="xin", bufs=2) as xpool,
            tc.tile_pool(name="hbuf", bufs=1) as hpool,
            tc.tile_pool(name="qkv", bufs=1) as qkvpool,
            tc.tile_pool(name="expb", bufs=12) as epool,
            tc.tile_pool(name="usb", bufs=1) as upool,
            tc.tile_pool(name="outb", bufs=1) as opool,
            tc.tile_pool(name="stat", bufs=2) as spool,
            tc.tile_pool(name="mm512", bufs=4, space="PSUM") as mmps,
            tc.tile_pool(name="ups", bufs=2, space="PSUM") as ups,
            tc.tile_pool(name="sums", bufs=1, space="PSUM") as sums,
            tc.tile_pool(name="statps", bufs=1, space="PSUM") as statps,
        ):
            # ---- one-time loads (tiny params first; big weights are
            # streamed in usage order AFTER x so the stats/h chain and the
            # first q matmuls are not stuck behind 4MB of weight DMA) ----
            qb_t = wpool.tile([128, 2 * CT], F32, tag="qb")
            nc.sync.dma_start(out=qb_t, in_=qb_d[:, :])
            nw_t = wpool.tile([128, CT], F32, tag="nw")
            nc.sync.dma_start(out=nw_t, in_=nw_d[:, :])
            nb_t = wpool.tile([128, CT], F32, tag="nb")
            nc.sync.dma_start(out=nb_t, in_=nb_d[:, :])
            pbe_t = wpool.tile([128, CT], F32, tag="pbe")
            nc.sync.dma_start(out=pbe_t, in_=pbe_d[:, :])
            e_t = wpool.tile([128, 8], F32, tag="E")
            nc.sync.dma_start(out=e_t, in_=e_d[:, :])
            et_t = wpool.tile([8, 128], F32, tag="ET")
            nc.sync.dma_start(out=et_t, in_=et_d[:, :])
            ones_f = wpool.tile([128, 1], F32, tag="onesf")
            nc.vector.memset(ones_f, 1.0)
            ones_t = wpool.tile([128, 1], F32R, tag="ones")
            nc.vector.tensor_copy(out=ones_t, in_=ones_f)
            eps_t = wpool.tile([8, 1], F32, tag="eps")
            nc.vector.memset(eps_t, EPS)
            # pre-load ACT function tables so the first real Sqrt/Exp/
            # Identity doesn't eat a 1.3us ACT_TABLE_LOAD on the critical path
            warm_t = wpool.tile([1, 4], F32, tag="warm")
            nc.vector.memset(warm_t, 1.0)
            for wf in (
                mybir.ActivationFunctionType.Sqrt,
                mybir.ActivationFunctionType.Exp,
                mybir.ActivationFunctionType.Identity,
            ):
                nc.scalar.activation(
                    out=warm_t[:, 1:2], in_=warm_t[:, 0:1], func=wf,
                    bias=0.0, scale=1.0,
                )
            wq_t = wpool.tile([128, CT, C], F32R, tag="wq")
            wk_t = wpool.tile([128, CT, C], F32R, tag="wk")
            wv_t = wpool.tile([128, CT, C], F32R, tag="wv")
            wp_t = wpool.tile([128, CT, C], F32R, tag="wp")

            x_dma_insts: list = []

            def load_weight(dst, dram):
                w_inst = nc.sync.dma_start(out=dst, in_=dram[:, :, :])
                # keep batch-0 x DMAs ahead of weight bytes in the queues
                for xi in x_dma_insts[:2]:
                    add_dep_helper(w_inst.ins, xi.ins, sync=True)

            def stats_a(b, fast=False):
                """x load + per-channel stats + group reduce + finalize.

                fast=True (first batch): per-channel sums via ACT accum and
                sum-of-squares via DVE reduce, running the two engines in
                parallel instead of the serial bn_stats chain; scratch
                outputs land in the not-yet-used h/q buffer slots.
                """
                x_t = xpool.tile([128, CT, N], F32, tag="x")
                x_dma_insts.clear()
                for ct in range(CT):
                    x_dma_insts.append(
                        nc.sync.dma_start(
                            out=x_t[:, ct, :], in_=x_d[b][:, ct, :]
                        )
                    )
                mvall = spool.tile([128, CT, 2], F32, tag="mvall")
                if fast:
                    scr_a = hpool.tile([128, CT, N], F32, tag="h", name="scr_a")
                    scr_v = qkvpool.tile([128, CT, N], F32, tag="q", name="scr_v")
                    for ct in range(CT):
                        nc.scalar.activation(
                            out=scr_a[:, ct, :],
                            in_=x_t[:, ct, :],
                            func=mybir.ActivationFunctionType.Square,
                            accum_out=mvall[:, ct, 1:2],
                        )
                        nc.vector.tensor_scalar(
                            out=scr_v[:, ct, :],
                            in0=x_t[:, ct, :],
                            scalar1=0.0,
                            scalar2=0.0,
                            op0=mybir.AluOpType.add,
                            op1=mybir.AluOpType.add,
                            accum_out=mvall[:, ct, 0:1],
                        )
                else:
                    mvsq = spool.tile([128, CT], F32, tag="mvsq")
                    for ct in range(CT):
                        st = spool.tile([128, 2, 6], F32, tag="bnstats")
                        for h2 in range(2):
                            nc.vector.bn_stats(
                                out=st[:, h2, :],
                                in_=x_t[:, ct, h2 * 512 : (h2 + 1) * 512],
                            )
                        nc.vector.bn_aggr(out=mvall[:, ct, :], in_=st)
                    # mvall[:, :, 1] = var + mean^2 = E[x^2]
                    nc.vector.tensor_mul(mvsq, mvall[:, :, 0], mvall[:, :, 0])
                    nc.vector.tensor_add(mvall[:, :, 1], mvall[:, :, 1], mvsq)
                # group sums over 16-partition groups
                gsum_ps = statps.tile([8, CT, 2], F32, tag="statps")
                for ct in range(CT):
                    nc.tensor.matmul(
                        out=gsum_ps[:, ct, :],
                        lhsT=e_t,
                        rhs=mvall[:, ct, :],
                        start=True,
                        stop=True,
                    )
                s_t = spool.tile([8, CT, 2], F32, tag="sstat")
                nc.scalar.mul(
                    out=s_t, in_=gsum_ps, mul=(1.0 / (GS * N)) if fast else 1.0 / GS
                )
                ssq = spool.tile([8, CT], F32, tag="ssq")
                nc.vector.tensor_mul(ssq, s_t[:, :, 0], s_t[:, :, 0])
                nc.vector.tensor_sub(s_t[:, :, 1], s_t[:, :, 1], ssq)
                nc.scalar.activation(
                    out=s_t[:, :, 1],
                    in_=s_t[:, :, 1],
                    func=mybir.ActivationFunctionType.Sqrt,
                    bias=eps_t,
                    scale=1.0,
                )
                nc.vector.reciprocal(out=s_t[:, :, 1], in_=s_t[:, :, 1])
                return x_t, s_t

            def stats_b(x_t, s_t):
                """broadcast (mean, rstd) to channels; h = x*A + B (f32r)."""
                bc_ps = statps.tile([128, CT, 2], F32, tag="statps")
                for ct in range(CT):
                    nc.tensor.matmul(
                        out=bc_ps[:, ct, :],
                        lhsT=et_t,
                        rhs=s_t[:, ct, :],
                        start=True,
                        stop=True,
                    )
                ab_t = spool.tile([128, 2, CT], F32, tag="ab")
                tmp_t = spool.tile([128, CT], F32, tag="abtmp")
                for ct in range(CT):
                    nc.vector.tensor_mul(
                        ab_t[:, 0, ct : ct + 1],
                        bc_ps[:, ct, 1:2],
                        nw_t[:, ct : ct + 1],
                    )
                    nc.vector.tensor_scalar_mul(
                        tmp_t[:, ct : ct + 1],
                        bc_ps[:, ct, 0:1],
                        ab_t[:, 0, ct : ct + 1],
                    )
                    nc.vector.tensor_sub(
                        ab_t[:, 1, ct : ct + 1],
                        nb_t[:, ct : ct + 1],
                        tmp_t[:, ct : ct + 1],
                    )
                h_t = hpool.tile([128, CT, N], F32R, tag="h")
                for ct in range(CT):
                    if ct < 2:
                        nc.scalar.activation(
                            out=h_t[:, ct, :],
                            in_=x_t[:, ct, :],
                            func=mybir.ActivationFunctionType.Identity,
                            bias=ab_t[:, 1, ct : ct + 1],
                            scale=ab_t[:, 0, ct : ct + 1],
                        )
                    else:
                        nc.vector.tensor_scalar(
                            out=h_t[:, ct, :],
                            in0=x_t[:, ct, :],
                            scalar1=ab_t[:, 0, ct : ct + 1],
                            scalar2=ab_t[:, 1, ct : ct + 1],
                            op0=mybir.AluOpType.mult,
                            op1=mybir.AluOpType.add,
                        )
                return h_t

            st = stats_a(0, fast=True)
            # weights stream behind x, in first-use order
            load_weight(wk_t, wk_d)
            load_weight(wq_t, wq_d)
            load_weight(wv_t, wv_d)
            load_weight(wp_t, wp_d)
            cur_x, cur_h = st[0], stats_b(*st)
            next_ctx: list = [None]

            for b in range(B_CORE):
                x_t, h_t = cur_x, cur_h

                # ---- q, k projections: [c part, i free] ----
                q_t = qkvpool.tile([128, CT, N], F32R, tag="q")
                k_t = qkvpool.tile([128, CT, N], F32R, tag="k")

                def qkv_chunk(dst, w_t, ct, ih, bias_col, on_act):
                    isl = slice(ih * 512, (ih + 1) * 512)
                    pp = mmps.tile([128, 512], F32, tag="mm", name="pp")
                    for kt in range(CT):
                        nc.tensor.matmul(
                            out=pp,
                            lhsT=w_t[:, kt, ct * 128 : (ct + 1) * 128],
                            rhs=h_t[:, kt, isl],
                            start=(kt == 0),
                            stop=(kt == CT - 1),
                        )
                    if on_act and ct < 2:
                        nc.scalar.activation(
                            out=dst[:, ct, isl],
                            in_=pp,
                            func=mybir.ActivationFunctionType.Identity,
                            bias=qb_t[:, bias_col : bias_col + 1],
                            scale=1.0,
                        )
                    else:
                        # bias-add + f32r round on DVE, keeping ACT free
                        nc.vector.tensor_scalar_add(
                            dst[:, ct, isl], pp, qb_t[:, bias_col : bias_col + 1]
                        )

                # k fully (scores need all j columns); q first half only —
                # the second half is produced during the ih0 score loop
                for ct in range(CT):
                    for ih in range(IH):
                        qkv_chunk(k_t, wk_t, ct, ih, CT + ct, on_act=False)
                for ct in range(CT):
                    qkv_chunk(q_t, wq_t, ct, 0, ct, on_act=True)
                # ---- v in transposed layout: vT [j part, c free] ----
                vt_t = qkvpool.tile([128, JT, C], F32R, tag="vt")
                for jt in range(JT):
                    vp = mmps.tile([128, 512], F32, tag="mm")
                    for kt in range(CT):
                        nc.tensor.matmul(
                            out=vp,
                            lhsT=h_t[:, kt, jt * 128 : (jt + 1) * 128],
                            rhs=wv_t[:, kt, :],
                            start=(kt == 0),
                            stop=(kt == CT - 1),
                        )
                    nc.vector.tensor_copy(out=vt_t[:, jt, :], in_=vp)

                # ---- attention ----
                out_t = opool.tile([128, CT, N], F32, tag="out")
                for ih in range(IH):
                    last_stage = ih == IH - 1
                    isl = slice(ih * 512, (ih + 1) * 512)
                    se_ps = sums.tile([1, 512], F32, tag="se")
                    e_list = []
                    for jt in range(JT):
                        sc_ps = mmps.tile([128, 512], F32, tag="mm")
                        for kt in range(CT):
                            nc.tensor.matmul(
                                out=sc_ps,
                                lhsT=k_t[:, kt, jt * 128 : (jt + 1) * 128],
                                rhs=q_t[:, kt, isl],
                                start=(kt == 0),
                                stop=(kt == CT - 1),
                            )
                        e_sb = epool.tile([128, 512], F32R, tag="exp")
                        nc.scalar.activation(
                            out=e_sb,
                            in_=sc_ps,
                            func=mybir.ActivationFunctionType.Exp,
                            bias=0.0,
                            scale=SCALE,
                        )
                        e_list.append(e_sb)
                        if jt >= 2:
                            nc.tensor.matmul(
                                out=se_ps,
                                lhsT=ones_t,
                                rhs=e_list[jt - 2],
                                start=(jt == 2),
                                stop=False,
                            )
                        # hoist deferred work into this PE-dense stretch:
                        # ih0: produce q's second half; ih1: next batch stats
                        if ih == 0 and 1 <= jt <= CT:
                            qkv_chunk(q_t, wq_t, jt - 1, 1, jt - 1, on_act=True)
                        if last_stage and b + 1 < B_CORE:
                            if jt == 3:
                                next_ctx[0] = stats_a(b + 1)
                            elif jt == 6:
                                nx, ns = next_ctx[0]
                                next_ctx[0] = (nx, stats_b(nx, ns))
                    for jtl in (JT - 2, JT - 1):
                        nc.tensor.matmul(
                            out=se_ps,
                            lhsT=ones_t,
                            rhs=e_list[jtl],
                            start=False,
                            stop=(jtl == JT - 1),
                        )
                    # ct-outer AV accumulation; casts overlap next ct's MMs.
                    # The [1,512] reciprocal (3.3us on one partition) is
                    # split into [1,128] chunks interleaved between the
                    # casts so it never head-of-line-blocks the DVE FIFO.
                    rec_sb = upool.tile([1, 512], F32, tag="rec")
                    rb_t = upool.tile([128, 512], F32, tag="rb")
                    u_sb = upool.tile([128, CT, 512], F32R, tag="usb")
                    for ct in range(CT):
                        u_ps = ups.tile([128, 512], F32, tag="u")
                        for jt in range(JT):
                            nc.tensor.matmul(
                                out=u_ps,
                                lhsT=vt_t[:, jt, ct * 128 : (ct + 1) * 128],
                                rhs=e_list[jt],
                                start=(jt == 0),
                                stop=(jt == JT - 1),
                            )
                        # cast on ACT (idle during AV) so the last cast
                        # follows its AV matmul immediately; recip chunks on
                        # DVE run concurrently
                        nc.scalar.copy(out=u_sb[:, ct, :], in_=u_ps)
                        csl = slice(ct * 128, (ct + 1) * 128)
                        nc.vector.reciprocal(
                            out=rec_sb[:, csl], in_=se_ps[:, csl]
                        )
                        nc.gpsimd.partition_broadcast(
                            rb_t[:, csl], rec_sb[:, csl]
                        )
                    # ---- proj + epilogue ----
                    for ct in range(CT):
                        f_ps = mmps.tile([128, 512], F32, tag="mm")
                        for kt in range(CT):
                            nc.tensor.matmul(
                                out=f_ps,
                                lhsT=wp_t[:, kt, ct * 128 : (ct + 1) * 128],
                                rhs=u_sb[:, kt, :],
                                start=(kt == 0),
                                stop=(kt == CT - 1),
                            )
                        m_sb = epool.tile([128, 512], F32, tag="exp")
                        nc.vector.tensor_mul(m_sb, f_ps, rb_t)
                        nc.vector.scalar_tensor_tensor(
                            out=out_t[:, ct, isl],
                            in0=m_sb,
                            scalar=pbe_t[:, ct : ct + 1],
                            in1=x_t[:, ct, isl],
                            op0=mybir.AluOpType.add,
                            op1=mybir.AluOpType.add,
                        )
                    if b == B_CORE - 1:
                        # last batch: fire each half as soon as it's done
                        nc.sync.dma_start(
                            out=out_d[b][:, :, isl], in_=out_t[:, :, isl]
                        )
                if b < B_CORE - 1:
                    nc.sync.dma_start(out=out_d[b][:, :, :], in_=out_t)
                if b + 1 < B_CORE:
                    cur_x, cur_h = next_ctx[0]
                    next_ctx[0] = None
    nc.compile()
    return nc


_NC_CACHE: list[bass.Bass | None] = [None]


def kernel(
    x: np.ndarray,
    norm_w: np.ndarray,
    norm_b: np.ndarray,
    qkv_w: np.ndarray,
    qkv_b: np.ndarray,
    proj_w: np.ndarray,
    proj_b: np.ndarray,
) -> np.ndarray:
    in_maps = prepare_in_maps(x, norm_w, norm_b, qkv_w, qkv_b, proj_w, proj_b)
    if _NC_CACHE[0] is None:
        _NC_CACHE[0] = build_program()
    nc = _NC_CACHE[0]
    res = run_bass_kernel_spmd(nc, in_maps, list(range(N_CORES)))
    return assemble_output(res.results, x.shape)


def prepare_in_maps(x, norm_w, norm_b, qkv_w, qkv_b, proj_w, proj_b):
    x = np.ascontiguousarray(np.asarray(x, dtype=np.float32))
    norm_w = np.asarray(norm_w, dtype=np.float32)
    norm_b = np.asarray(norm_b, dtype=np.float32)
    qkv_w = np.asarray(qkv_w, dtype=np.float32)
    qkv_b = np.asarray(qkv_b, dtype=np.float32)
    proj_w = np.asarray(proj_w, dtype=np.float32)
    proj_b = np.asarray(proj_b, dtype=np.float32)

    Bf, Cf, H, W = x.shape
    assert (Bf, Cf, H * W) == (B_TOTAL, C, N)

    # weights pre-transposed + packed (128, CT, 512) so each SBUF partition
    # loads one contiguous 8KB run, and pre-rounded to fp32r
    wT = round_fp32r(np.ascontiguousarray(qkv_w.T))  # (C_in, 3C)
    wT = wT.reshape(CT, 128, 3 * C).transpose(1, 0, 2)  # (128, CT, 3C)
    wq = np.ascontiguousarray(wT[:, :, :C])
    wk = np.ascontiguousarray(wT[:, :, C : 2 * C])
    wv = np.ascontiguousarray(wT[:, :, 2 * C :])
    wpT = round_fp32r(np.ascontiguousarray(proj_w.T))
    wp = np.ascontiguousarray(wpT.reshape(CT, 128, C).transpose(1, 0, 2))

    qb_t = np.ascontiguousarray(qkv_b[: 2 * C].reshape(2 * CT, 128).T)
    nw_t = np.ascontiguousarray(norm_w.reshape(CT, 128).T)
    nb_t = np.ascontiguousarray(norm_b.reshape(CT, 128).T)
    pb_eff = (
        proj_b.astype(np.float64) + proj_w.astype(np.float64) @ qkv_b[2 * C :]
    ).astype(np.float32)
    pbe_t = np.ascontiguousarray(pb_eff.reshape(CT, 128).T)
    pidx = np.arange(128)
    E = (pidx[:, None] // GS == np.arange(8)[None, :]).astype(np.float32)
    ET = np.ascontiguousarray(E.T)

    # x packed (B, 128, CT, N): 16KB contiguous per partition per batch
    xp = np.ascontiguousarray(
        x.reshape(B_TOTAL, CT, 128, N).transpose(0, 2, 1, 3)
    )
    shared = {
        "wq": wq,
        "wk": wk,
        "wv": wv,
        "wp": wp,
        "qb_t": qb_t,
        "nw_t": nw_t,
        "nb_t": nb_t,
        "pbe_t": pbe_t,
        "E": E,
        "ET": ET,
    }
    return [
        {"x": np.ascontiguousarray(xp[c * B_CORE : (c + 1) * B_CORE]), **shared}
        for c in range(N_CORES)
    ]


def assemble_output(results, x_shape):
    Bf, Cf, H, W = x_shape
    out = np.concatenate([results[c]["out"] for c in range(N_CORES)], axis=0)
    # (B, 128, CT, N) -> (B, C, H, W)
    out = out.transpose(0, 2, 1, 3).reshape(B_TOTAL, C, H, W)
    return np.ascontiguousarray(out)
